# revision 1
# baseline (speedup 1.0000x reference)
"""GAT (2-layer, 8-head) fused Bass kernel for 8 trn2 NeuronCores.

Sharding: nodes (rows of x) split 512/core. Layer-1 h/s computed locally per
core, AllGather'd (h+ones in bf16, scores in fp32); each core computes its
512xN attention block for all 8 heads; layer-1 output xc (+ its layer-2
projection, ones and scores) AllGather'd again (fp32); each core computes its
512xN layer-2 attention block and the final log_softmax rows.

Key algebra: with s_i = h_i . a_src, d_j = h_j . a_dst,
  exp(leakyrelu(s_i + d_j)) = max(exp(s_i)exp(d_j), exp(.2 s_i)exp(.2 d_j))
and softmax over j is invariant to any per-i scale, so the attention
numerator can be taken as P[i,j] = max(b_j, w_i * dd_j) with
  b_j = exp(d_j), w_i = exp(-0.8 s_i), dd_j = exp(0.2 d_j).
One DVE/GPSIMD tensor_scalar (mult, max) per [128,512] tile; exp only on
vectors. elu(x) = max(x, min(exp(x)-1, 0)); log_softmax via Exp(accum_out)+Ln.
Matmuls run as float32r (1 cyc/row) or bf16; fp32 would be 4 cyc/row.
"""

import numpy as np

N, NFEAT, NHID, NCLASS, NHEADS = 4096, 512, 64, 16, 8
NC = 8                      # cores
NQ = N // NC                # 512 own nodes per core
QT = NQ // 128              # 4 query tiles per core
JT = N // 128               # 32 key tiles
ALPHA = 0.2
HW = NHID * NHEADS          # 512
HXC = NHEADS * (NHID + 1)   # 520: per-head 64 h cols + ones col (bf16 AG1)
AGC2 = 18                   # AG2: 16 outh + 1 ones + 1 sdst2

_CACHE = {}


def _build_nc(no_cc=False, no_l1=False):
    import concourse.bass as bass
    import concourse.bacc as bacc
    import concourse.mybir as mybir
    import concourse.tile as tile
    from concourse.masks import make_identity

    fp32 = mybir.dt.float32
    fp32r = mybir.dt.float32r
    bf16 = mybir.dt.bfloat16
    AX = mybir.AxisListType.X
    OP = mybir.AluOpType
    AF = mybir.ActivationFunctionType

    nc = bacc.Bacc()
    xT = nc.declare_dram_parameter("xT", [NFEAT, NQ], fp32, isOutput=False)
    Whr = nc.declare_dram_parameter("Whr", [NFEAT, HW], fp32, isOutput=False)
    Asd = nc.declare_dram_parameter("Asd", [NFEAT, 16], fp32, isOutput=False)
    Wo = nc.declare_dram_parameter("Wo", [HW, NCLASS], fp32, isOutput=False)
    aod = nc.declare_dram_parameter("aod", [2, NCLASS], fp32, isOutput=False)
    out = nc.declare_dram_parameter("out", [NQ, NCLASS], fp32, isOutput=True)

    with tile.TileContext(nc) as tc:
        with (
            tc.tile_pool(name="const", bufs=1) as constp,
            tc.tile_pool(name="big", bufs=1) as bigp,
            tc.tile_pool(name="work", bufs=3) as workp,
            tc.tile_pool(name="pp", bufs=8) as ppool,
            tc.tile_pool(name="ps_acc", bufs=3, space="PSUM") as ps_acc,
            tc.tile_pool(name="ps_t", bufs=4, space="PSUM") as ps_t,
            tc.tile_pool(name="dram", bufs=1, space="DRAM") as dramp,
        ):
            v, sc, g, te, dma = nc.vector, nc.scalar, nc.gpsimd, nc.tensor, nc.sync

            ident = constp.tile([128, 128], fp32, tag="ident")
            make_identity(nc, ident[:])
            # sel[k, h*128+m] = 1 iff k == h: one-hot row selector for
            # partition-broadcast matmuls (out = sel_h.T @ rows)
            self_f = constp.tile([8, 8 * 128], fp32, tag="self_f")
            g.memset(self_f[:], 0.0)
            g.affine_select(
                out=self_f[:].rearrange("k (h m) -> k h m", m=128),
                in_=self_f[:].rearrange("k (h m) -> k h m", m=128),
                compare_op=mybir.AluOpType.not_equal,
                fill=1.0, base=0, channel_multiplier=1,
                pattern=[[-1, 8], [0, 128]])
            sel = constp.tile([8, 8 * 128], fp32r, tag="sel")
            v.tensor_copy(sel[:], self_f[:])

            # ---- A. load params (fp32 load + fp32r cast for PE use) ----
            xT_sb, whr_sb, asd_sb, wo_sb = [], [], [], []
            for k in range(4):
                tf = workp.tile([128, NQ], fp32, tag="pload", name=f"xTf{k}")
                dma.dma_start(tf[:], xT[k * 128:(k + 1) * 128, :])
                t = constp.tile([128, NQ], fp32r, tag=f"xT{k}", name=f"xT{k}")
                v.tensor_copy(t[:], tf[:])
                xT_sb.append(t)
                tf = workp.tile([128, HW], fp32, tag="pload", name=f"whrf{k}")
                dma.dma_start(tf[:], Whr[k * 128:(k + 1) * 128, :])
                t = constp.tile([128, HW], fp32r, tag=f"whr{k}", name=f"whr{k}")
                v.tensor_copy(t[:], tf[:])
                whr_sb.append(t)
                tf = workp.tile([128, 16], fp32, tag="pload16", name=f"asdf{k}")
                dma.dma_start(tf[:], Asd[k * 128:(k + 1) * 128, :])
                t = constp.tile([128, 16], fp32r, tag=f"asd{k}", name=f"asd{k}")
                v.tensor_copy(t[:], tf[:])
                asd_sb.append(t)
                t = constp.tile([128, 16], fp32, tag=f"wo{k}", name=f"wo{k}")
                dma.dma_start(t[:], Wo[k * 128:(k + 1) * 128, :])
                wo_sb.append(t)
            aos_b = constp.tile([128, 16], fp32, tag="aos_b")
            dma.dma_start(aos_b[:], aod[0:1, :].to_broadcast((128, 16)))
            aod_b = constp.tile([128, 16], fp32, tag="aod_b")
            dma.dma_start(aod_b[:], aod[1:2, :].to_broadcast((128, 16)))

            ag1h_in = dramp.tile([NQ, HXC], bf16, tag="ag1h_in")
            ag1h_out = dramp.tile([N, HXC], bf16, tag="ag1h_out",
                                  addr_space="Local" if no_cc else "Shared")
            ag1s_in = dramp.tile([NQ, 16], fp32, tag="ag1s_in")
            ag1s_out = dramp.tile([N, 16], fp32, tag="ag1s_out",
                                  addr_space="Local" if no_cc else "Shared")
            ag2_in = dramp.tile([NQ, AGC2], fp32, tag="ag2_in")
            ag2_out = dramp.tile([N, AGC2], fp32, tag="ag2_out",
                                 addr_space="Local" if no_cc else "Shared")

            # ---- B. h_ownT (feat-major) ----
            hT_sb = []
            for f in range(4):
                ps = ps_acc.tile([128, NQ], fp32, tag="acc")
                for k in range(4):
                    te.matmul(ps[:], whr_sb[k][:, f * 128:(f + 1) * 128],
                              xT_sb[k][:], start=(k == 0), stop=(k == 3))
                t = constp.tile([128, NQ], fp32r, tag=f"hT{f}", name=f"hT{f}")
                (v.tensor_copy if f % 2 else sc.copy)(t[:], ps[:])
                hT_sb.append(t)

            # ---- D. s_own [16, NQ] rows 0:8 src, 8:16 dst ----
            s_ps = ps_acc.tile([16, NQ], fp32, tag="acc")
            for k in range(4):
                te.matmul(s_ps[:], asd_sb[k][:], hT_sb[k][:],
                          start=(k == 0), stop=(k == 3))
            s_sb = constp.tile([16, NQ], fp32, tag="s_sb")
            v.tensor_copy(s_sb[:], s_ps[:])

            # ---- F. w panel: exp(-0.8 * s_src), bcast via K=1 matmul ----
            w_sb = constp.tile([NHEADS, NQ], fp32r, tag="w_sb")
            sc.activation(w_sb[:], s_sb[0:NHEADS, :], AF.Exp, scale=-0.8)
            wb = []
            for h in range(NHEADS):
                bp = ps_t.tile([128, NQ], fp32, tag="bigtp", bufs=1)
                te.matmul(bp[:], sel[:, h * 128:(h + 1) * 128],
                          w_sb[:], start=True, stop=True)
                t = constp.tile([128, NQ], bf16, tag=f"wb{h}", name=f"wb{h}")
                (v.tensor_copy if h % 2 else sc.copy)(t[:], bp[:])
                wb.append(t)

            # ---- C/E. h_own + stage AG1 (h bf16 + s fp32) ----
            for qt in range(QT):
                ps = ps_acc.tile([128, HW], fp32, tag="acc")
                for k in range(4):
                    te.matmul(ps[:], xT_sb[k][:, qt * 128:(qt + 1) * 128],
                              whr_sb[k][:], start=(k == 0), stop=(k == 3))
                stg = workp.tile([128, HXC], bf16, tag="stage")
                sc.copy(stg[:].rearrange("p (h c) -> p h c", c=65)[:, :, 0:64],
                        ps[:].rearrange("p (h c) -> p h c", c=64))
                g.memset(
                    stg[:].rearrange("p (h c) -> p h c", c=65)[:, :, 64:65], 1.0)
                dma.dma_start(ag1h_in[qt * 128:(qt + 1) * 128, :], stg[:])
                tp = ps_t.tile([128, 16], fp32, tag="tp")
                te.transpose(tp[:], s_sb[:, qt * 128:(qt + 1) * 128],
                             ident[0:16, 0:16])
                stgs = workp.tile([128, 16], fp32, tag="stgs")
                v.tensor_copy(stgs[:], tp[:])
                dma.dma_start(ag1s_in[qt * 128:(qt + 1) * 128, :], stgs[:])

            # ---- G. AllGather 1 (both buffers in one op) ----
            if no_cc:
                for r in range(NC):
                    dma.dma_start(ag1h_out[r * NQ:(r + 1) * NQ, :], ag1h_in[:])
                    dma.dma_start(ag1s_out[r * NQ:(r + 1) * NQ, :], ag1s_in[:])
            else:
                g.collective_compute(
                    "AllGather", OP.bypass,
                    ins=[ag1s_in.opt()], outs=[ag1s_out.opt()],
                    replica_groups=[list(range(NC))],
                )
                g.collective_compute(
                    "AllGather", OP.bypass,
                    ins=[ag1h_in.opt()], outs=[ag1h_out.opt()],
                    replica_groups=[list(range(NC))],
                )

            # ---- H. key-side score panels (bf16) ----
            sd_pan = constp.tile([128, JT * NHEADS], fp32, tag="sd_pan")
            dma.dma_start(
                sd_pan[:].rearrange("p (t h) -> p t h", h=NHEADS),
                ag1s_out[:, 8:16].rearrange("(t p) h -> p t h", p=128))
            b_all = constp.tile([128, JT * NHEADS], fp32, tag="b_all")
            sc.activation(b_all[:], sd_pan[:], AF.Exp)
            d_all = constp.tile([128, JT * NHEADS], fp32, tag="d_all")
            sc.activation(d_all[:], sd_pan[:], AF.Exp, scale=ALPHA)

            # ---- I. hx tiles (persistent keys, bf16) ----
            hx = []
            for jt in range(JT):
                t = bigp.tile([128, HXC], bf16, tag=f"hx{jt}", name=f"hx{jt}")
                dma.dma_start(t[:], ag1h_out[jt * 128:(jt + 1) * 128, :])
                hx.append(t)

            # ---- J/K. layer-1 attention ----
            xr = [bigp.tile([128, HW], fp32, tag=f"xr{qt}", name=f"xr{qt}")
                  for qt in range(QT)]
            xc_sb = [bigp.tile([128, HW], fp32, tag=f"xc{qt}", name=f"xc{qt}")
                     for qt in range(QT)]
            xcT_sb = [constp.tile([128, NQ], fp32, tag=f"xcT{f}", name=f"xcT{f}")
                      for f in range(4)]

            def elu_block(qt, fb):
                # elu on xr cols of head pair fb -> xc_sb, then transpose
                # into xcT_sb[fb] (overlaps with later heads' attention)
                c0, c1 = fb * 128, (fb + 1) * 128
                ex = workp.tile([128, 128], fp32, tag="ex")
                sc.activation(ex[:], xr[qt][:, c0:c1], AF.Exp)
                v.tensor_scalar(ex[:], ex[:], 1.0, 0.0, OP.subtract, OP.min)
                v.tensor_tensor(xc_sb[qt][:, c0:c1], xr[qt][:, c0:c1], ex[:],
                                OP.max)
                tp = ps_t.tile([128, 128], fp32, tag="tp")
                te.transpose(tp[:], xc_sb[qt][:, c0:c1], ident[:])
                eng_copy = sc.copy if fb % 2 else v.tensor_copy
                eng_copy(xcT_sb[fb][:, qt * 128:(qt + 1) * 128], tp[:])

            for h in range(NHEADS if not no_l1 else 0):
                acc = ps_acc.tile([65, NQ], fp32, tag="acc")
                for jt in range(JT):
                    pt = ppool.tile([128, NQ], bf16, tag="pt")
                    eng = g if (jt % 6 == 5) else v
                    eng.tensor_scalar(
                        pt[:], wb[h][:],
                        d_all[:, jt * NHEADS + h:jt * NHEADS + h + 1],
                        b_all[:, jt * NHEADS + h:jt * NHEADS + h + 1],
                        OP.mult, OP.max)
                    te.matmul(acc[:], hx[jt][:, h * 65:(h + 1) * 65], pt[:],
                              start=(jt == 0), stop=(jt == JT - 1))
                fT = workp.tile([65, NQ], fp32, tag="fT")
                sc.copy(fT[:], acc[:])
                den = workp.tile([128, QT], fp32, tag="den")
                tps = []
                for qt in range(QT):
                    tp = ps_t.tile([128, 65], fp32, tag="tp", name=f"tp{qt}")
                    te.transpose(tp[:], fT[:, qt * 128:(qt + 1) * 128],
                                 ident[0:65, 0:65])
                    sc.copy(den[:, qt:qt + 1], tp[:, 64:65])
                    tps.append(tp)
                r = workp.tile([128, QT], fp32, tag="recip")
                v.reciprocal(r[:], den[:])
                for qt in range(QT):
                    v.tensor_scalar(xr[qt][:, h * 64:(h + 1) * 64],
                                    tps[qt][:, 0:64], r[:, qt:qt + 1], None,
                                    OP.mult)
                if h % 2 == 1:
                    for qt in range(QT):
                        elu_block(qt, h // 2)

            # ---- K2/L fallback for no_l1 timing variant ----
            w2tmp = constp.tile([128, QT], fp32, tag="w2tmp")
            if no_l1:
                for qt in range(QT):
                    g.memset(xr[qt][:], 0.5)
                for qt in range(QT):
                    for fb in range(4):
                        elu_block(qt, fb)
            stg2s = [bigp.tile([128, AGC2], fp32, tag=f"stage2_{qt}",
                               name=f"stage2_{qt}") for qt in range(QT)]

            # ---- M. outh_own; scores; stage AG2 ----
            for qt in range(QT):
                ps = ps_t.tile([128, 16], fp32, tag="tp")
                for k in range(4):
                    te.matmul(ps[:], xcT_sb[k][:, qt * 128:(qt + 1) * 128],
                              wo_sb[k][:], start=(k == 0), stop=(k == 3))
                stg = stg2s[qt]
                v.tensor_copy(stg[:, 0:16], ps[:])
                g.memset(stg[:, 16:17], 1.0)
                tmp = workp.tile([128, 16], fp32, tag="sdtmp")
                v.tensor_tensor(tmp[:], ps[:], aod_b[:], OP.mult)
                v.tensor_reduce(stg[:, 17:18], tmp[:], AX, OP.add)
                v.tensor_tensor(tmp[:], ps[:], aos_b[:], OP.mult)
                v.tensor_reduce(w2tmp[:, qt:qt + 1], tmp[:], AX, OP.add)
                dma.dma_start(ag2_in[qt * 128:(qt + 1) * 128, :], stg[:])

            # ---- N. w2 bcast: [128,QT] -> row [1,NQ] -> bcast matmul ----
            w2e = constp.tile([128, QT], fp32, tag="w2e")
            sc.activation(w2e[:], w2tmp[:], AF.Exp, scale=-0.8)
            w2tp = ps_t.tile([QT, 128], fp32, tag="tp")
            te.transpose(w2tp[:], w2e[:], ident[:])
            w2tps = constp.tile([QT, 128], fp32r, tag="w2tps")
            v.tensor_copy(w2tps[:], w2tp[:])
            w2b = constp.tile([128, NQ], bf16, tag="w2b")
            for qt in range(QT):
                w2ps = ps_t.tile([128, 128], fp32, tag="tp")
                te.matmul(w2ps[:], sel[0:QT, qt * 128:(qt + 1) * 128],
                          w2tps[:], start=True, stop=True)
                sc.copy(w2b[:, qt * 128:(qt + 1) * 128], w2ps[:])

            # ---- O. AllGather 2 ----
            if no_cc:
                for r in range(NC):
                    dma.dma_start(ag2_out[r * NQ:(r + 1) * NQ, :], ag2_in[:])
            else:
                g.collective_compute(
                    "AllGather", OP.bypass,
                    ins=[ag2_in.opt()], outs=[ag2_out.opt()],
                    replica_groups=[list(range(NC))],
                )

            # ---- P. layer-2 panels ----
            hx2f = constp.tile([128, JT * 17], fp32, tag="hx2f")
            dma.dma_start(
                hx2f[:].rearrange("p (t c) -> p t c", c=17),
                ag2_out[:, 0:17].rearrange("(t p) c -> p t c", p=128))
            hx2 = constp.tile([128, JT * 17], bf16, tag="hx2")
            sc.copy(hx2[:], hx2f[:])
            sd2 = constp.tile([128, JT], fp32, tag="sd2")
            dma.dma_start(
                sd2[:].rearrange("p (t c) -> p t c", c=1),
                ag2_out[:, 17:18].rearrange("(t p) c -> p t c", p=128))
            b2 = constp.tile([128, JT], fp32, tag="b2")
            sc.activation(b2[:], sd2[:], AF.Exp)
            d2 = constp.tile([128, JT], fp32, tag="d2")
            sc.activation(d2[:], sd2[:], AF.Exp, scale=ALPHA)

            # ---- Q. layer-2 attention ----
            acc2 = ps_acc.tile([17, NQ], fp32, tag="acc")
            for jt in range(JT):
                pt = ppool.tile([128, NQ], bf16, tag="pt")
                eng = g if (jt % 6 == 5) else v
                eng.tensor_scalar(pt[:], w2b[:],
                                  d2[:, jt:jt + 1], b2[:, jt:jt + 1],
                                  OP.mult, OP.max)
                te.matmul(acc2[:], hx2[:, jt * 17:(jt + 1) * 17], pt[:],
                          start=(jt == 0), stop=(jt == JT - 1))
            f2 = workp.tile([17, NQ], fp32, tag="f2")
            sc.copy(f2[:], acc2[:])

            # ---- R. normalize, elu, log_softmax, store (ACT batched) ----
            den2 = workp.tile([128, QT], fp32, tag="den")
            t2s = []
            for qt in range(QT):
                tp = ps_t.tile([128, 17], fp32, tag="tp", name=f"t2_{qt}")
                te.transpose(tp[:], f2[:, qt * 128:(qt + 1) * 128],
                             ident[0:17, 0:17])
                sc.copy(den2[:, qt:qt + 1], tp[:, 16:17])
                t2s.append(tp)
            r2 = workp.tile([128, QT], fp32, tag="recip")
            v.reciprocal(r2[:], den2[:])
            os_, eos, elus, ses = [], [], [], []
            for qt in range(QT):
                o = workp.tile([128, NCLASS], fp32, tag=f"o{qt}", name=f"o{qt}")
                v.tensor_scalar(o[:], t2s[qt][:, 0:16], r2[:, qt:qt + 1], None,
                                OP.mult)
                os_.append(o)
            for qt in range(QT):
                eo = workp.tile([128, NCLASS], fp32, tag=f"eo{qt}",
                                name=f"eo{qt}")
                sc.activation(eo[:], os_[qt][:], AF.Exp)
                eos.append(eo)
            for qt in range(QT):
                v.tensor_scalar(eos[qt][:], eos[qt][:], 1.0, 0.0,
                                OP.subtract, OP.min)
                elu = workp.tile([128, NCLASS], fp32, tag=f"elu{qt}",
                                 name=f"elu{qt}")
                v.tensor_tensor(elu[:], os_[qt][:], eos[qt][:], OP.max)
                elus.append(elu)
            for qt in range(QT):
                se = workp.tile([128, 1], fp32, tag=f"se{qt}", name=f"se{qt}")
                e2 = workp.tile([128, NCLASS], fp32, tag="e2")
                sc.activation(e2[:], elus[qt][:], AF.Exp, accum_out=se[:])
                ses.append(se)
            lses = []
            for qt in range(QT):
                lse = workp.tile([128, 1], fp32, tag=f"lse{qt}",
                                 name=f"lse{qt}")
                sc.activation(lse[:], ses[qt][:], AF.Ln)
                lses.append(lse)
            for qt in range(QT):
                fin = workp.tile([128, NCLASS], fp32, tag="fin")
                v.tensor_scalar(fin[:], elus[qt][:], lses[qt][:], None,
                                OP.subtract)
                dma.dma_start(out[qt * 128:(qt + 1) * 128, :], fin[:])

    nc.finalize()
    return nc


def _get_compiled(no_cc=False, no_l1=False):
    key = ("nc", no_cc, no_l1)
    if key not in _CACHE:
        _CACHE[key] = _build_nc(no_cc=no_cc, no_l1=no_l1)
    return _CACHE[key]


def kernel(x, Wh, ah, Wo, ao):
    from concourse.bass_utils import run_bass_kernel_spmd

    nc = _get_compiled()
    x = np.asarray(x, np.float32)
    Wh = np.asarray(Wh, np.float32)
    ah = np.asarray(ah, np.float32)
    Wo = np.asarray(Wo, np.float32)
    ao = np.asarray(ao, np.float32)

    # host-side relayouts (no math): head-major weight matrix, block-diag
    # score matrix, split ao
    Whr = np.ascontiguousarray(
        Wh.transpose(1, 0, 2).reshape(NFEAT, HW))          # [512, 512]
    Asd = np.zeros((NFEAT, 16), np.float32)
    for h in range(NHEADS):
        Asd[h * NHID:(h + 1) * NHID, h] = ah[h, :NHID]      # src
        Asd[h * NHID:(h + 1) * NHID, 8 + h] = ah[h, NHID:]  # dst
    aod = np.stack([ao[:NCLASS], ao[NCLASS:]])              # [2, 16]

    in_maps = []
    for i in range(NC):
        in_maps.append({
            "xT": np.ascontiguousarray(x[i * NQ:(i + 1) * NQ].T),
            "Whr": Whr, "Asd": Asd,
            "Wo": np.ascontiguousarray(Wo), "aod": aod,
        })
    res = run_bass_kernel_spmd(nc, in_maps, list(range(NC)))
    return np.concatenate([res.results[i]["out"] for i in range(NC)], 0)



# revision 8
# speedup vs baseline: 1.4431x; 1.4431x over previous
"""GAT (2-layer, 8-head) fused Bass kernel for 8 trn2 NeuronCores.

Sharding: nodes (rows of x) split 512/core. Layer-1 h/scores computed locally
per core and AllGather'd (h+ones bf16 in two 4-head chunks for overlap,
scores fp32); each core computes its 512xN attention block for all 8 heads;
layer-2 projection + scores AllGather'd (fp32); each core computes its 512xN
layer-2 block and the final log_softmax rows.

Key algebra: with s_i = h_i . a_src, d_j = h_j . a_dst,
  exp(leakyrelu(s_i + d_j)) = max(exp(s_i)exp(d_j), exp(.2 s_i)exp(.2 d_j))
and softmax over j is invariant to any per-i scale, so the attention
numerator is P[k,q] = max(b_k, w_q * dd_k) with b = exp(d), w = exp(-.8 s),
dd = exp(.2 d).  P tiles are built key-major [128k, 512q] so w is the tensor
operand and b/dd are per-partition scalars; they are consumed as matmul
weights (lhsT) against the gathered h (rhs, 65 cols incl. a ones column for
the denominator), so attention output lands query-major [128q, 65] and needs
no transposes.  P builds are spread across DVE (tensor_scalar mult+max),
Activation (relu(dd*w - b), plus a rank-1 " + sum_k b_k h_k" PE fixup) and
GPSIMD.  s is computed as x @ (W a) with host-precomputed Wa.
"""

import numpy as np

N, NFEAT, NHID, NCLASS, NHEADS = 4096, 512, 64, 16, 8
NC = 8                      # cores
NQ = N // NC                # 512 own nodes per core
QT = NQ // 128              # 4 query tiles per core
JT = N // 128               # 32 key tiles
HW = NHID * NHEADS          # 512
HC = 4 * (NHID + 1)         # 260: AG1h chunk cols (4 heads x (64 h + ones))
AGC2 = 18                   # AG2: 16 outh + 1 ones + 1 sdst2

# per-head P-tile engine assignment: v=DVE, a=Act(relu form), g=GPSIMD
_ASSIGN = []
for _jt in range(JT):
    if _jt % 8 == 4:
        _ASSIGN.append("g")
    elif _jt % 6 == 2:
        _ASSIGN.append("a")
    else:
        _ASSIGN.append("v")

_CACHE = {}


def _build_nc(no_cc=False):
    import concourse.bass as bass
    import concourse.bacc as bacc
    import concourse.mybir as mybir
    import concourse.tile as tile
    from concourse.masks import make_identity

    fp32 = mybir.dt.float32
    bf16 = mybir.dt.bfloat16
    AX = mybir.AxisListType.X
    OP = mybir.AluOpType
    AF = mybir.ActivationFunctionType

    nc = bacc.Bacc()
    xT = nc.declare_dram_parameter("xT", [NFEAT, NQ], bf16, isOutput=False)
    Whr = nc.declare_dram_parameter("Whr", [NFEAT, HW], bf16, isOutput=False)
    Wa = nc.declare_dram_parameter("Wa", [NFEAT, 16], bf16, isOutput=False)
    Wo = nc.declare_dram_parameter("Wo", [HW, NCLASS], bf16, isOutput=False)
    aod = nc.declare_dram_parameter("aod", [2, NCLASS], fp32, isOutput=False)
    out = nc.declare_dram_parameter("out", [NQ, NCLASS], fp32, isOutput=True)

    with tile.TileContext(nc) as tc:
        with (
            tc.tile_pool(name="const", bufs=1) as constp,
            tc.tile_pool(name="work", bufs=3) as workp,
            tc.tile_pool(name="pp", bufs=10) as ppool,
            tc.tile_pool(name="ps_attn", bufs=3, space="PSUM") as ps_attn,
            tc.tile_pool(name="ps_b", bufs=2, space="PSUM") as ps_b,
            tc.tile_pool(name="ps_t", bufs=2, space="PSUM") as ps_t,
            tc.tile_pool(name="ps_s", bufs=1, space="PSUM") as ps_s,
            tc.tile_pool(name="dram", bufs=1, space="DRAM") as dramp,
        ):
            v, sc, g, te, dma = nc.vector, nc.scalar, nc.gpsimd, nc.tensor, nc.sync

            ident = constp.tile([128, 128], bf16, tag="ident")
            make_identity(nc, ident[:])
            ident_f = constp.tile([128, 128], fp32, tag="ident_f")
            make_identity(nc, ident_f[:])
            # sel[k, h*128+m] = 1 iff k == h: one-hot row selector for
            # partition-broadcast matmuls (out = sel_h.T @ rows)
            sel = constp.tile([8, 8 * 128], bf16, tag="sel")
            g.memset(sel[:], 0.0)
            g.affine_select(
                out=sel[:].rearrange("k (h m) -> k h m", m=128),
                in_=sel[:].rearrange("k (h m) -> k h m", m=128),
                compare_op=OP.not_equal,
                fill=1.0, base=0, channel_multiplier=1,
                pattern=[[-1, 8], [0, 128]])
            ones_row = constp.tile([1, 128], bf16, tag="ones_row")
            g.memset(ones_row[:], 1.0)

            # ---- A. param loads (bf16, batched) ----
            xt = constp.tile([128, 4 * NQ], bf16, tag="xt")
            dma.dma_start(xt[:].rearrange("p (k q) -> p k q", k=4),
                          xT.rearrange("(k p) q -> p k q", p=128))
            wh = constp.tile([128, 4 * HW], bf16, tag="wh")
            dma.dma_start(wh[:].rearrange("p (k c) -> p k c", k=4),
                          Whr.rearrange("(k p) c -> p k c", p=128))
            wa = constp.tile([128, 64], bf16, tag="wa")
            dma.dma_start(wa[:].rearrange("p (k c) -> p k c", k=4),
                          Wa.rearrange("(k p) c -> p k c", p=128))
            wo = constp.tile([128, 64], bf16, tag="wo")
            dma.dma_start(wo[:].rearrange("p (k c) -> p k c", k=4),
                          Wo.rearrange("(k p) c -> p k c", p=128))
            aos_b = constp.tile([128, 16], fp32, tag="aos_b")
            dma.dma_start(aos_b[:], aod[0:1, :].to_broadcast((128, 16)))
            aod_b = constp.tile([128, 16], fp32, tag="aod_b")
            dma.dma_start(aod_b[:], aod[1:2, :].to_broadcast((128, 16)))

            ag1s_in = dramp.tile([NQ, 16], fp32, tag="ag1s_in")
            ag1s_out = dramp.tile([N, 16], fp32, tag="ag1s_out",
                                  addr_space="Local" if no_cc else "Shared")
            agh_in = [dramp.tile([NQ, HC], bf16, tag=f"agh_in{c}",
                                 name=f"agh_in{c}") for c in range(2)]
            agh_out = [dramp.tile([N, HC], bf16, tag=f"agh_out{c}",
                                  name=f"agh_out{c}",
                                  addr_space="Local" if no_cc else "Shared")
                       for c in range(2)]
            ag2_in = dramp.tile([NQ, AGC2], fp32, tag="ag2_in")
            ag2_out = dramp.tile([N, AGC2], fp32, tag="ag2_out",
                                 addr_space="Local" if no_cc else "Shared")

            # ---- B. h_own + s_own per query tile; stage AG1 ----
            stg = [constp.tile([128, QT * HC], bf16, tag=f"stg{c}",
                               name=f"stg{c}") for c in range(2)]
            s_own = constp.tile([128, QT * 16], fp32, tag="s_own")
            for qt in range(QT):
                ps_h = ps_b.tile([128, HW], fp32, tag="bp")
                for k in range(4):
                    te.matmul(ps_h[:],
                              xt[:, k * NQ + qt * 128:k * NQ + (qt + 1) * 128],
                              wh[:, k * HW:(k + 1) * HW],
                              start=(k == 0), stop=(k == 3))
                pss = ps_s.tile([128, 16], fp32, tag="sp")
                for k in range(4):
                    te.matmul(pss[:],
                              xt[:, k * NQ + qt * 128:k * NQ + (qt + 1) * 128],
                              wa[:, k * 16:(k + 1) * 16],
                              start=(k == 0), stop=(k == 3))
                for c in range(2):
                    sc.copy(
                        stg[c][:, qt * HC:(qt + 1) * HC]
                        .rearrange("p (h c) -> p h c", c=65)[:, :, 0:64],
                        ps_h[:, c * 256:(c + 1) * 256]
                        .rearrange("p (h c) -> p h c", c=64))
                v.tensor_copy(s_own[:, qt * 16:(qt + 1) * 16], pss[:])
            for c in range(2):
                g.memset(
                    stg[c][:].rearrange("p (q h c) -> p q h c", h=4, c=65)
                    [:, :, :, 64:65], 1.0)

            # ---- C. stage DMAs + collectives ----
            dma.dma_start(ag1s_in.rearrange("(q p) c -> p q c", p=128),
                          s_own[:].rearrange("p (q c) -> p q c", c=16))
            for c in range(2):
                dma.dma_start(agh_in[c].rearrange("(q p) c -> p q c", p=128),
                              stg[c][:].rearrange("p (q c) -> p q c", c=HC))
            if not no_cc:
                g.collective_compute(
                    "AllGather", OP.bypass,
                    ins=[ag1s_in.opt()], outs=[ag1s_out.opt()],
                    replica_groups=[list(range(NC))])
                for c in range(2):
                    g.collective_compute(
                        "AllGather", OP.bypass,
                        ins=[agh_in[c].opt()], outs=[agh_out[c].opt()],
                        replica_groups=[list(range(NC))])

            # ---- D. gathered score panels: b = exp(d), dd = exp(.2 d) ----
            sd_pan = constp.tile([128, JT * 16], fp32, tag="sd_pan")
            for r in range(NC):
                src = (ag1s_in if no_cc
                       else ag1s_out[r * NQ:(r + 1) * NQ, :])
                dma.dma_start(
                    sd_pan[:, r * 64:(r + 1) * 64]
                    .rearrange("p (t c) -> p t c", c=16),
                    src.rearrange("(t p) c -> p t c", p=128))
            dstv = sd_pan[:].rearrange("p (t c) -> p t c", c=16)[:, :, 8:16]
            b_all = constp.tile([128, JT * NHEADS], fp32, tag="b_all")
            sc.activation(b_all[:].rearrange("p (t h) -> p t h", h=8), dstv,
                          AF.Exp)
            d_all = constp.tile([128, JT * NHEADS], fp32, tag="d_all")
            sc.activation(d_all[:].rearrange("p (t h) -> p t h", h=8), dstv,
                          AF.Exp, scale=0.2)
            negb = constp.tile([128, JT * NHEADS], fp32, tag="negb")
            v.tensor_scalar(negb[:], b_all[:], -1.0, None, OP.mult)
            b_bf = constp.tile([128, JT * NHEADS], bf16, tag="b_bf")
            g.tensor_copy(b_bf[:], b_all[:])

            # ---- E. hx loads (per-replica reads, 2 chunks of 4 heads) ----
            hxc = []
            for c in range(2):
                hx = constp.tile([128, JT * HC], bf16, tag=f"hx{c}",
                                 name=f"hx{c}")
                for r in range(NC):
                    src = (agh_in[c] if no_cc
                           else agh_out[c][r * NQ:(r + 1) * NQ, :])
                    dma.dma_start(
                        hx[:, r * 4 * HC:(r + 1) * 4 * HC]
                        .rearrange("p (t c) -> p t c", c=HC),
                        src.rearrange("(t p) c -> p t c", p=128))
                hxc.append(hx)

            # ---- F. w panel: exp(-0.8 s_src) bcast to [128, NQ] per head ----
            s_f = constp.tile([16, NQ], fp32, tag="s_f")
            for qt in range(QT):
                tp = ps_t.tile([16, 128], fp32, tag="tp", name="tp_s")
                te.transpose(tp[:], s_own[:, qt * 16:(qt + 1) * 16],
                             ident_f[:])
                g.tensor_copy(s_f[:, qt * 128:(qt + 1) * 128], tp[:])
            w8 = constp.tile([8, NQ], bf16, tag="w8")
            sc.activation(w8[:], s_f[0:8, :], AF.Exp, scale=-0.8)
            wb = []
            for h in range(NHEADS):
                bp = ps_b.tile([128, NQ], fp32, tag="bp")
                te.matmul(bp[:], sel[:, h * 128:(h + 1) * 128], w8[:],
                          start=True, stop=True)
                t = constp.tile([128, NQ], bf16, tag=f"wb{h}", name=f"wb{h}")
                (v.tensor_copy, sc.copy, g.tensor_copy)[h % 3](t[:], bp[:])
                wb.append(t)

            # ---- G. layer-1 attention (flipped: out is query-major) ----
            xr = [constp.tile([128, HW], bf16, tag=f"xr{qt}", name=f"xr{qt}")
                  for qt in range(QT)]
            for h in range(NHEADS):
                hx = hxc[h // 4]
                coff = (h % 4) * 65
                acc = ps_attn.tile([128, QT * 65], fp32, tag="acc")
                n_act = sum(1 for a in _ASSIGN if a == "a")
                accC = None
                if n_act:
                    accC = ps_s.tile([1, 65], fp32, tag="sp", name="accC")
                seen_act = 0
                for jt in range(JT):
                    col = jt * NHEADS + h
                    eng = _ASSIGN[jt]
                    pt = ppool.tile([128, NQ], bf16, tag="pt")
                    if eng == "a":
                        sc.activation(pt[:], wb[h][:], AF.Relu,
                                      bias=negb[:, col:col + 1],
                                      scale=d_all[:, col:col + 1])
                        te.matmul(accC[:], b_bf[:, col:col + 1],
                                  hx[:, jt * HC + coff:jt * HC + coff + 65],
                                  start=(seen_act == 0),
                                  stop=(seen_act == n_act - 1))
                        seen_act += 1
                    else:
                        e = v if eng == "v" else g
                        e.tensor_scalar(pt[:], wb[h][:],
                                        d_all[:, col:col + 1],
                                        b_all[:, col:col + 1],
                                        OP.mult, OP.max)
                    for qt in range(QT):
                        te.matmul(acc[:, qt * 65:(qt + 1) * 65],
                                  pt[:, qt * 128:(qt + 1) * 128],
                                  hx[:, jt * HC + coff:jt * HC + coff + 65],
                                  start=(jt == 0),
                                  stop=(jt == JT - 1 and n_act == 0))
                if n_act:
                    crow = workp.tile([1, 65], bf16, tag="crow")
                    g.tensor_copy(crow[:], accC[:])
                    for qt in range(QT):
                        te.matmul(acc[:, qt * 65:(qt + 1) * 65], ones_row[:],
                                  crow[:], start=False, stop=True)
                # normalize: den fp32 from PSUM, feat via bf16 copy
                racc = workp.tile([128, QT], fp32, tag="racc")
                v.reciprocal(
                    racc[:],
                    acc[:].rearrange("p (q c) -> p q c", c=65)[:, :, 64])
                ab = workp.tile([128, QT * 65], bf16, tag="ab")
                sc.copy(ab[:], acc[:])
                for qt in range(QT):
                    v.tensor_scalar(xr[qt][:, h * 64:(h + 1) * 64],
                                    ab[:, qt * 65:qt * 65 + 64],
                                    racc[:, qt:qt + 1], None, OP.mult)

            # ---- H. elu -> xc (bf16) -> xcT ----
            xc = [constp.tile([128, HW], bf16, tag=f"xc{qt}", name=f"xc{qt}")
                  for qt in range(QT)]
            xcT = [constp.tile([128, NQ], bf16, tag=f"xcT{k}", name=f"xcT{k}")
                   for k in range(4)]
            for qt in range(QT):
                ex = workp.tile([128, HW], bf16, tag="ex")
                sc.activation(ex[:], xr[qt][:], AF.Exp)
                v.tensor_scalar(ex[:], ex[:], 1.0, 0.0, OP.subtract, OP.min)
                v.tensor_tensor(xc[qt][:], xr[qt][:], ex[:], OP.max)
            for qt in range(QT):
                for fb in range(4):
                    tp = ps_t.tile([128, 128], bf16, tag="tp")
                    te.transpose(tp[:], xc[qt][:, fb * 128:(fb + 1) * 128],
                                 ident[:])
                    e = (v.tensor_copy, sc.copy, g.tensor_copy)[(qt * 4 + fb) % 3]
                    e(xcT[fb][:, qt * 128:(qt + 1) * 128], tp[:])

            # ---- I. layer-2 projection, scores, stage AG2 ----
            stg2 = constp.tile([128, QT * AGC2], fp32, tag="stg2")
            w2tmp = workp.tile([128, QT], fp32, tag="w2tmp")
            for qt in range(QT):
                pso = ps_s.tile([128, 16], fp32, tag="sp")
                for k in range(4):
                    te.matmul(pso[:], xcT[k][:, qt * 128:(qt + 1) * 128],
                              wo[:, k * 16:(k + 1) * 16],
                              start=(k == 0), stop=(k == 3))
                v.tensor_copy(stg2[:, qt * AGC2:qt * AGC2 + 16], pso[:])
                tmp = workp.tile([128, 16], fp32, tag="sdtmp")
                v.tensor_tensor(tmp[:], pso[:], aod_b[:], OP.mult)
                v.tensor_reduce(stg2[:, qt * AGC2 + 17:qt * AGC2 + 18],
                                tmp[:], AX, OP.add)
                v.tensor_tensor(tmp[:], pso[:], aos_b[:], OP.mult)
                v.tensor_reduce(w2tmp[:, qt:qt + 1], tmp[:], AX, OP.add)
            g.memset(
                stg2[:].rearrange("p (q c) -> p q c", c=AGC2)[:, :, 16:17],
                1.0)
            dma.dma_start(ag2_in.rearrange("(q p) c -> p q c", p=128),
                          stg2[:].rearrange("p (q c) -> p q c", c=AGC2))
            if not no_cc:
                g.collective_compute(
                    "AllGather", OP.bypass,
                    ins=[ag2_in.opt()], outs=[ag2_out.opt()],
                    replica_groups=[list(range(NC))])

            # ---- J. w2 panel ----
            w2e = workp.tile([128, QT], fp32, tag="w2e")
            sc.activation(w2e[:], w2tmp[:], AF.Exp, scale=-0.8)
            w2tp = ps_t.tile([QT, 128], fp32, tag="tp", name="w2tp")
            te.transpose(w2tp[:], w2e[:], ident_f[:])
            w2s = workp.tile([QT, 128], bf16, tag="w2s")
            v.tensor_copy(w2s[:], w2tp[:])
            w2b = constp.tile([128, NQ], bf16, tag="w2b")
            for qt in range(QT):
                w2ps = ps_t.tile([128, 128], fp32, tag="tp")
                te.matmul(w2ps[:], sel[0:QT, qt * 128:(qt + 1) * 128],
                          w2s[:], start=True, stop=True)
                (sc.copy if qt % 2 else v.tensor_copy)(
                    w2b[:, qt * 128:(qt + 1) * 128], w2ps[:])

            # ---- K. layer-2 panels ----
            hx2f = constp.tile([128, JT * AGC2], fp32, tag="hx2f")
            for r in range(NC):
                src = (ag2_in if no_cc
                       else ag2_out[r * NQ:(r + 1) * NQ, :])
                dma.dma_start(
                    hx2f[:, r * 4 * AGC2:(r + 1) * 4 * AGC2]
                    .rearrange("p (t c) -> p t c", c=AGC2),
                    src.rearrange("(t p) c -> p t c", p=128))
            hx2 = constp.tile([128, JT * 17], bf16, tag="hx2")
            sc.copy(hx2[:].rearrange("p (t c) -> p t c", c=17),
                    hx2f[:].rearrange("p (t c) -> p t c", c=AGC2)[:, :, 0:17])
            sd2 = hx2f[:].rearrange("p (t c) -> p t c", c=AGC2)[:, :, 17]
            b2 = constp.tile([128, JT], fp32, tag="b2")
            sc.activation(b2[:], sd2, AF.Exp)
            d2 = constp.tile([128, JT], fp32, tag="d2")
            sc.activation(d2[:], sd2, AF.Exp, scale=0.2)

            # ---- L. layer-2 attention ----
            acc2 = ps_attn.tile([128, QT * 17], fp32, tag="acc")
            for jt in range(JT):
                pt = ppool.tile([128, NQ], bf16, tag="pt")
                e = g if (jt % 8 == 4) else v
                e.tensor_scalar(pt[:], w2b[:], d2[:, jt:jt + 1],
                                b2[:, jt:jt + 1], OP.mult, OP.max)
                for qt in range(QT):
                    te.matmul(acc2[:, qt * 17:(qt + 1) * 17],
                              pt[:, qt * 128:(qt + 1) * 128],
                              hx2[:, jt * 17:(jt + 1) * 17],
                              start=(jt == 0), stop=(jt == JT - 1))

            # ---- M. normalize, elu, log_softmax, store (fp32 epilogue) ----
            r2 = workp.tile([128, QT], fp32, tag="r2")
            v.reciprocal(
                r2[:], acc2[:].rearrange("p (q c) -> p q c", c=17)[:, :, 16])
            o4 = workp.tile([128, QT * 16], fp32, tag="o4")
            for qt in range(QT):
                v.tensor_scalar(o4[:, qt * 16:(qt + 1) * 16],
                                acc2[:, qt * 17:qt * 17 + 16],
                                r2[:, qt:qt + 1], None, OP.mult)
            ex2 = workp.tile([128, QT * 16], fp32, tag="ex2")
            sc.activation(ex2[:], o4[:], AF.Exp)
            v.tensor_scalar(ex2[:], ex2[:], 1.0, 0.0, OP.subtract, OP.min)
            z2 = workp.tile([128, QT * 16], fp32, tag="z2")
            v.tensor_tensor(z2[:], o4[:], ex2[:], OP.max)
            fin = workp.tile([128, QT * 16], fp32, tag="fin")
            scratch = workp.tile([128, 16], fp32, tag="scr")
            for qt in range(QT):
                se = workp.tile([128, 1], fp32, tag=f"se{qt}", name=f"se{qt}")
                sc.activation(scratch[:], z2[:, qt * 16:(qt + 1) * 16],
                              AF.Exp, accum_out=se[:])
                lse = workp.tile([128, 1], fp32, tag=f"lse{qt}",
                                 name=f"lse{qt}")
                sc.activation(lse[:], se[:], AF.Ln)
                v.tensor_scalar(fin[:, qt * 16:(qt + 1) * 16],
                                z2[:, qt * 16:(qt + 1) * 16],
                                lse[:], None, OP.subtract)
            dma.dma_start(out.rearrange("(q p) c -> p q c", p=128),
                          fin[:].rearrange("p (q c) -> p q c", c=16))

    nc.finalize()
    return nc


def _get_compiled(no_cc=False):
    key = ("nc", no_cc)
    if key not in _CACHE:
        _CACHE[key] = _build_nc(no_cc=no_cc)
    return _CACHE[key]


def kernel(x, Wh, ah, Wo, ao):
    from concourse.bass_utils import run_bass_kernel_spmd
    import ml_dtypes

    bf = ml_dtypes.bfloat16
    nc = _get_compiled()
    x = np.asarray(x, np.float32)
    Wh = np.asarray(Wh, np.float32)
    ah = np.asarray(ah, np.float32)
    Wo = np.asarray(Wo, np.float32)
    ao = np.asarray(ao, np.float32)

    # host-side relayouts (no device math): head-major weight matrix,
    # score projection Wa = Wh @ a per head (src cols 0:8, dst cols 8:16)
    Whr = np.ascontiguousarray(
        Wh.transpose(1, 0, 2).reshape(NFEAT, HW)).astype(bf)   # [512, 512]
    Wa = np.zeros((NFEAT, 16), np.float32)
    for h in range(NHEADS):
        Wa[:, h] = Wh[h] @ ah[h, :NHID]
        Wa[:, 8 + h] = Wh[h] @ ah[h, NHID:]
    Wa = Wa.astype(bf)
    aodm = np.stack([ao[:NCLASS], ao[NCLASS:]])                # [2, 16]

    in_maps = []
    for i in range(NC):
        in_maps.append({
            "xT": np.ascontiguousarray(x[i * NQ:(i + 1) * NQ].T).astype(bf),
            "Whr": Whr, "Wa": Wa,
            "Wo": np.ascontiguousarray(Wo).astype(bf), "aod": aodm,
        })
    res = run_bass_kernel_spmd(nc, in_maps, list(range(NC)))
    return np.concatenate([res.results[i]["out"] for i in range(NC)], 0)


# revision 9
# speedup vs baseline: 1.4432x; 1.0001x over previous
"""GAT (2-layer, 8-head) fused Bass kernel for 8 trn2 NeuronCores.

Sharding: nodes (rows of x) split 512/core. Layer-1 h/scores computed locally
per core and AllGather'd (h+ones bf16 in two 4-head chunks for overlap,
scores fp32); each core computes its 512xN attention block for all 8 heads;
layer-2 projection + scores AllGather'd (fp32); each core computes its 512xN
layer-2 block and the final log_softmax rows.

Key algebra: with s_i = h_i . a_src, d_j = h_j . a_dst,
  exp(leakyrelu(s_i + d_j)) = max(exp(s_i)exp(d_j), exp(.2 s_i)exp(.2 d_j))
and softmax over j is invariant to any per-i scale, so the attention
numerator is P[k,q] = max(b_k, w_q * dd_k) with b = exp(d), w = exp(-.8 s),
dd = exp(.2 d).  P tiles are built key-major [128k, 512q] so w is the tensor
operand and b/dd are per-partition scalars; they are consumed as matmul
weights (lhsT) against the gathered h (rhs, 65 cols incl. a ones column for
the denominator), so attention output lands query-major [128q, 65] and needs
no transposes.  P builds are spread across DVE (tensor_scalar mult+max),
Activation (relu(dd*w - b), plus a rank-1 " + sum_k b_k h_k" PE fixup) and
GPSIMD.  s is computed as x @ (W a) with host-precomputed Wa.
"""

import numpy as np

N, NFEAT, NHID, NCLASS, NHEADS = 4096, 512, 64, 16, 8
NC = 8                      # cores
NQ = N // NC                # 512 own nodes per core
QT = NQ // 128              # 4 query tiles per core
JT = N // 128               # 32 key tiles
HW = NHID * NHEADS          # 512
HC = 4 * (NHID + 1)         # 260: AG1h chunk cols (4 heads x (64 h + ones))
AGC2 = 18                   # AG2: 16 outh + 1 ones + 1 sdst2

# per-head P-tile engine assignment: v=DVE, a=Act(relu form), g=GPSIMD
_ASSIGN = []
for _jt in range(JT):
    if _jt % 8 == 4:
        _ASSIGN.append("g")
    elif _jt % 6 == 2:
        _ASSIGN.append("a")
    else:
        _ASSIGN.append("v")

_CACHE = {}


def _build_nc(no_cc=False):
    import concourse.bass as bass
    import concourse.bacc as bacc
    import concourse.mybir as mybir
    import concourse.tile as tile
    from concourse.masks import make_identity

    fp32 = mybir.dt.float32
    bf16 = mybir.dt.bfloat16
    AX = mybir.AxisListType.X
    OP = mybir.AluOpType
    AF = mybir.ActivationFunctionType

    nc = bacc.Bacc()
    xT = nc.declare_dram_parameter("xT", [NFEAT, NQ], bf16, isOutput=False)
    Whr = nc.declare_dram_parameter("Whr", [NFEAT, HW], bf16, isOutput=False)
    Wa = nc.declare_dram_parameter("Wa", [NFEAT, 16], bf16, isOutput=False)
    Wo = nc.declare_dram_parameter("Wo", [HW, NCLASS], bf16, isOutput=False)
    aod = nc.declare_dram_parameter("aod", [2, NCLASS], fp32, isOutput=False)
    out = nc.declare_dram_parameter("out", [NQ, NCLASS], fp32, isOutput=True)

    with tile.TileContext(nc) as tc:
        with (
            tc.tile_pool(name="const", bufs=1) as constp,
            tc.tile_pool(name="work", bufs=3) as workp,
            tc.tile_pool(name="pp", bufs=10) as ppool,
            tc.tile_pool(name="ps_attn", bufs=3, space="PSUM") as ps_attn,
            tc.tile_pool(name="ps_b", bufs=2, space="PSUM") as ps_b,
            tc.tile_pool(name="ps_t", bufs=2, space="PSUM") as ps_t,
            tc.tile_pool(name="ps_s", bufs=1, space="PSUM") as ps_s,
            tc.tile_pool(name="dram", bufs=1, space="DRAM") as dramp,
        ):
            v, sc, g, te, dma = nc.vector, nc.scalar, nc.gpsimd, nc.tensor, nc.sync

            ident = constp.tile([128, 128], bf16, tag="ident")
            make_identity(nc, ident[:])
            ident_f = constp.tile([128, 128], fp32, tag="ident_f")
            make_identity(nc, ident_f[:])
            # sel[k, h*128+m] = 1 iff k == h: one-hot row selector for
            # partition-broadcast matmuls (out = sel_h.T @ rows)
            sel = constp.tile([8, 8 * 128], bf16, tag="sel")
            g.memset(sel[:], 0.0)
            g.affine_select(
                out=sel[:].rearrange("k (h m) -> k h m", m=128),
                in_=sel[:].rearrange("k (h m) -> k h m", m=128),
                compare_op=OP.not_equal,
                fill=1.0, base=0, channel_multiplier=1,
                pattern=[[-1, 8], [0, 128]])
            ones_row = constp.tile([1, 128], bf16, tag="ones_row")
            g.memset(ones_row[:], 1.0)

            # ---- A. param loads (bf16, batched) ----
            xt = constp.tile([128, 4 * NQ], bf16, tag="xt")
            dma.dma_start(xt[:].rearrange("p (k q) -> p k q", k=4),
                          xT.rearrange("(k p) q -> p k q", p=128))
            wh = constp.tile([128, 4 * HW], bf16, tag="wh")
            dma.dma_start(wh[:].rearrange("p (k c) -> p k c", k=4),
                          Whr.rearrange("(k p) c -> p k c", p=128))
            wa = constp.tile([128, 64], bf16, tag="wa")
            dma.dma_start(wa[:].rearrange("p (k c) -> p k c", k=4),
                          Wa.rearrange("(k p) c -> p k c", p=128))
            wo = constp.tile([128, 64], bf16, tag="wo")
            dma.dma_start(wo[:].rearrange("p (k c) -> p k c", k=4),
                          Wo.rearrange("(k p) c -> p k c", p=128))
            aos_b = constp.tile([128, 16], fp32, tag="aos_b")
            dma.dma_start(aos_b[:], aod[0:1, :].to_broadcast((128, 16)))
            aod_b = constp.tile([128, 16], fp32, tag="aod_b")
            dma.dma_start(aod_b[:], aod[1:2, :].to_broadcast((128, 16)))

            ag1s_in = dramp.tile([NQ, 16], fp32, tag="ag1s_in")
            ag1s_out = dramp.tile([N, 16], fp32, tag="ag1s_out",
                                  addr_space="Local" if no_cc else "Shared")
            agh_in = [dramp.tile([NQ, HC], bf16, tag=f"agh_in{c}",
                                 name=f"agh_in{c}") for c in range(2)]
            agh_out = [dramp.tile([N, HC], bf16, tag=f"agh_out{c}",
                                  name=f"agh_out{c}",
                                  addr_space="Local" if no_cc else "Shared")
                       for c in range(2)]
            ag2_in = dramp.tile([NQ, AGC2], fp32, tag="ag2_in")
            ag2_out = dramp.tile([N, AGC2], fp32, tag="ag2_out",
                                 addr_space="Local" if no_cc else "Shared")

            # ---- B. h_own + s_own per query tile; stage AG1 ----
            stg = [constp.tile([128, QT * HC], bf16, tag=f"stg{c}",
                               name=f"stg{c}") for c in range(2)]
            s_own = constp.tile([128, QT * 16], fp32, tag="s_own")
            for qt in range(QT):
                ps_h = ps_b.tile([128, HW], fp32, tag="bp")
                for k in range(4):
                    te.matmul(ps_h[:],
                              xt[:, k * NQ + qt * 128:k * NQ + (qt + 1) * 128],
                              wh[:, k * HW:(k + 1) * HW],
                              start=(k == 0), stop=(k == 3))
                pss = ps_s.tile([128, 16], fp32, tag="sp")
                for k in range(4):
                    te.matmul(pss[:],
                              xt[:, k * NQ + qt * 128:k * NQ + (qt + 1) * 128],
                              wa[:, k * 16:(k + 1) * 16],
                              start=(k == 0), stop=(k == 3))
                for c in range(2):
                    sc.copy(
                        stg[c][:, qt * HC:(qt + 1) * HC]
                        .rearrange("p (h c) -> p h c", c=65)[:, :, 0:64],
                        ps_h[:, c * 256:(c + 1) * 256]
                        .rearrange("p (h c) -> p h c", c=64))
                v.tensor_copy(s_own[:, qt * 16:(qt + 1) * 16], pss[:])
            for c in range(2):
                g.memset(
                    stg[c][:].rearrange("p (q h c) -> p q h c", h=4, c=65)
                    [:, :, :, 64:65], 1.0)

            # ---- C. stage DMAs + collectives ----
            dma.dma_start(ag1s_in.rearrange("(q p) c -> p q c", p=128),
                          s_own[:].rearrange("p (q c) -> p q c", c=16))
            for c in range(2):
                dma.dma_start(agh_in[c].rearrange("(q p) c -> p q c", p=128),
                              stg[c][:].rearrange("p (q c) -> p q c", c=HC))
            if not no_cc:
                g.collective_compute(
                    "AllGather", OP.bypass,
                    ins=[ag1s_in.opt()], outs=[ag1s_out.opt()],
                    replica_groups=[list(range(NC))])
                for c in range(2):
                    g.collective_compute(
                        "AllGather", OP.bypass,
                        ins=[agh_in[c].opt()], outs=[agh_out[c].opt()],
                        replica_groups=[list(range(NC))])

            # ---- D. gathered score panels: b = exp(d), dd = exp(.2 d) ----
            sd_pan = constp.tile([128, JT * 16], fp32, tag="sd_pan")
            for r in range(NC):
                src = (ag1s_in if no_cc
                       else ag1s_out[r * NQ:(r + 1) * NQ, :])
                dma.dma_start(
                    sd_pan[:, r * 64:(r + 1) * 64]
                    .rearrange("p (t c) -> p t c", c=16),
                    src.rearrange("(t p) c -> p t c", p=128))
            dstv = sd_pan[:].rearrange("p (t c) -> p t c", c=16)[:, :, 8:16]
            b_all = constp.tile([128, JT * NHEADS], fp32, tag="b_all")
            sc.activation(b_all[:].rearrange("p (t h) -> p t h", h=8), dstv,
                          AF.Exp)
            d_all = constp.tile([128, JT * NHEADS], fp32, tag="d_all")
            sc.activation(d_all[:].rearrange("p (t h) -> p t h", h=8), dstv,
                          AF.Exp, scale=0.2)
            negb = constp.tile([128, JT * NHEADS], fp32, tag="negb")
            v.tensor_scalar(negb[:], b_all[:], -1.0, None, OP.mult)
            b_bf = constp.tile([128, JT * NHEADS], bf16, tag="b_bf")
            g.tensor_copy(b_bf[:], b_all[:])

            # ---- E. hx loads (per-replica reads, 2 chunks of 4 heads) ----
            hxc = []
            for c in range(2):
                hx = constp.tile([128, JT * HC], bf16, tag=f"hx{c}",
                                 name=f"hx{c}")
                for r in range(NC):
                    src = (agh_in[c] if no_cc
                           else agh_out[c][r * NQ:(r + 1) * NQ, :])
                    dma.dma_start(
                        hx[:, r * 4 * HC:(r + 1) * 4 * HC]
                        .rearrange("p (t c) -> p t c", c=HC),
                        src.rearrange("(t p) c -> p t c", p=128))
                hxc.append(hx)

            # ---- F. w panel: exp(-0.8 s_src) bcast to [128, NQ] per head ----
            s_f = constp.tile([16, NQ], fp32, tag="s_f")
            for qt in range(QT):
                tp = ps_t.tile([16, 128], fp32, tag="tp", name="tp_s")
                te.transpose(tp[:], s_own[:, qt * 16:(qt + 1) * 16],
                             ident_f[:])
                (v.tensor_copy if qt % 2 else sc.copy)(
                    s_f[:, qt * 128:(qt + 1) * 128], tp[:])
            w8 = constp.tile([8, NQ], bf16, tag="w8")
            sc.activation(w8[:], s_f[0:8, :], AF.Exp, scale=-0.8)
            wb = []
            for h in range(NHEADS):
                bp = ps_b.tile([128, NQ], fp32, tag="bp")
                te.matmul(bp[:], sel[:, h * 128:(h + 1) * 128], w8[:],
                          start=True, stop=True)
                t = constp.tile([128, NQ], bf16, tag=f"wb{h}", name=f"wb{h}")
                (v.tensor_copy if h % 2 else sc.copy)(t[:], bp[:])
                wb.append(t)

            # ---- G. layer-1 attention (flipped: out is query-major) ----
            xr = [constp.tile([128, HW], bf16, tag=f"xr{qt}", name=f"xr{qt}")
                  for qt in range(QT)]
            for h in range(NHEADS):
                hx = hxc[h // 4]
                coff = (h % 4) * 65
                acc = ps_attn.tile([128, QT * 65], fp32, tag="acc")
                n_act = sum(1 for a in _ASSIGN if a == "a")
                accC = None
                if n_act:
                    accC = ps_s.tile([1, 65], fp32, tag="sp", name="accC")
                seen_act = 0
                for jt in range(JT):
                    col = jt * NHEADS + h
                    eng = _ASSIGN[jt]
                    pt = ppool.tile([128, NQ], bf16, tag="pt")
                    if eng == "a":
                        sc.activation(pt[:], wb[h][:], AF.Relu,
                                      bias=negb[:, col:col + 1],
                                      scale=d_all[:, col:col + 1])
                        te.matmul(accC[:], b_bf[:, col:col + 1],
                                  hx[:, jt * HC + coff:jt * HC + coff + 65],
                                  start=(seen_act == 0),
                                  stop=(seen_act == n_act - 1))
                        seen_act += 1
                    else:
                        e = v if eng == "v" else g
                        e.tensor_scalar(pt[:], wb[h][:],
                                        d_all[:, col:col + 1],
                                        b_all[:, col:col + 1],
                                        OP.mult, OP.max)
                    for qt in range(QT):
                        te.matmul(acc[:, qt * 65:(qt + 1) * 65],
                                  pt[:, qt * 128:(qt + 1) * 128],
                                  hx[:, jt * HC + coff:jt * HC + coff + 65],
                                  start=(jt == 0),
                                  stop=(jt == JT - 1 and n_act == 0))
                if n_act:
                    crow = workp.tile([1, 65], bf16, tag="crow")
                    v.tensor_copy(crow[:], accC[:])
                    for qt in range(QT):
                        te.matmul(acc[:, qt * 65:(qt + 1) * 65], ones_row[:],
                                  crow[:], start=False, stop=True)
                # normalize: den fp32 from PSUM, feat via bf16 copy
                racc = workp.tile([128, QT], fp32, tag="racc")
                v.reciprocal(
                    racc[:],
                    acc[:].rearrange("p (q c) -> p q c", c=65)[:, :, 64])
                ab = workp.tile([128, QT * 65], bf16, tag="ab")
                sc.copy(ab[:], acc[:])
                for qt in range(QT):
                    v.tensor_scalar(xr[qt][:, h * 64:(h + 1) * 64],
                                    ab[:, qt * 65:qt * 65 + 64],
                                    racc[:, qt:qt + 1], None, OP.mult)

            # ---- H. elu -> xc (bf16) -> xcT ----
            xc = [constp.tile([128, HW], bf16, tag=f"xc{qt}", name=f"xc{qt}")
                  for qt in range(QT)]
            xcT = [constp.tile([128, NQ], bf16, tag=f"xcT{k}", name=f"xcT{k}")
                   for k in range(4)]
            for qt in range(QT):
                ex = workp.tile([128, HW], bf16, tag="ex")
                sc.activation(ex[:], xr[qt][:], AF.Exp)
                v.tensor_scalar(ex[:], ex[:], 1.0, 0.0, OP.subtract, OP.min)
                v.tensor_tensor(xc[qt][:], xr[qt][:], ex[:], OP.max)
            for qt in range(QT):
                for fb in range(4):
                    tp = ps_t.tile([128, 128], bf16, tag="tp")
                    te.transpose(tp[:], xc[qt][:, fb * 128:(fb + 1) * 128],
                                 ident[:])
                    e = v.tensor_copy if (qt * 4 + fb) % 2 else sc.copy
                    e(xcT[fb][:, qt * 128:(qt + 1) * 128], tp[:])

            # ---- I. layer-2 projection, scores, stage AG2 ----
            stg2 = constp.tile([128, QT * AGC2], fp32, tag="stg2")
            w2tmp = workp.tile([128, QT], fp32, tag="w2tmp")
            for qt in range(QT):
                pso = ps_s.tile([128, 16], fp32, tag="sp")
                for k in range(4):
                    te.matmul(pso[:], xcT[k][:, qt * 128:(qt + 1) * 128],
                              wo[:, k * 16:(k + 1) * 16],
                              start=(k == 0), stop=(k == 3))
                v.tensor_copy(stg2[:, qt * AGC2:qt * AGC2 + 16], pso[:])
                tmp = workp.tile([128, 16], fp32, tag="sdtmp")
                v.tensor_tensor(tmp[:], pso[:], aod_b[:], OP.mult)
                v.tensor_reduce(stg2[:, qt * AGC2 + 17:qt * AGC2 + 18],
                                tmp[:], AX, OP.add)
                v.tensor_tensor(tmp[:], pso[:], aos_b[:], OP.mult)
                v.tensor_reduce(w2tmp[:, qt:qt + 1], tmp[:], AX, OP.add)
            g.memset(
                stg2[:].rearrange("p (q c) -> p q c", c=AGC2)[:, :, 16:17],
                1.0)
            dma.dma_start(ag2_in.rearrange("(q p) c -> p q c", p=128),
                          stg2[:].rearrange("p (q c) -> p q c", c=AGC2))
            if not no_cc:
                g.collective_compute(
                    "AllGather", OP.bypass,
                    ins=[ag2_in.opt()], outs=[ag2_out.opt()],
                    replica_groups=[list(range(NC))])

            # ---- J. w2 panel ----
            w2e = workp.tile([128, QT], fp32, tag="w2e")
            sc.activation(w2e[:], w2tmp[:], AF.Exp, scale=-0.8)
            w2tp = ps_t.tile([QT, 128], fp32, tag="tp", name="w2tp")
            te.transpose(w2tp[:], w2e[:], ident_f[:])
            w2s = workp.tile([QT, 128], bf16, tag="w2s")
            v.tensor_copy(w2s[:], w2tp[:])
            w2b = constp.tile([128, NQ], bf16, tag="w2b")
            for qt in range(QT):
                w2ps = ps_t.tile([128, 128], fp32, tag="tp")
                te.matmul(w2ps[:], sel[0:QT, qt * 128:(qt + 1) * 128],
                          w2s[:], start=True, stop=True)
                (sc.copy if qt % 2 else v.tensor_copy)(
                    w2b[:, qt * 128:(qt + 1) * 128], w2ps[:])

            # ---- K. layer-2 panels ----
            hx2f = constp.tile([128, JT * AGC2], fp32, tag="hx2f")
            for r in range(NC):
                src = (ag2_in if no_cc
                       else ag2_out[r * NQ:(r + 1) * NQ, :])
                dma.dma_start(
                    hx2f[:, r * 4 * AGC2:(r + 1) * 4 * AGC2]
                    .rearrange("p (t c) -> p t c", c=AGC2),
                    src.rearrange("(t p) c -> p t c", p=128))
            hx2 = constp.tile([128, JT * 17], bf16, tag="hx2")
            sc.copy(hx2[:].rearrange("p (t c) -> p t c", c=17),
                    hx2f[:].rearrange("p (t c) -> p t c", c=AGC2)[:, :, 0:17])
            sd2 = hx2f[:].rearrange("p (t c) -> p t c", c=AGC2)[:, :, 17]
            b2 = constp.tile([128, JT], fp32, tag="b2")
            sc.activation(b2[:], sd2, AF.Exp)
            d2 = constp.tile([128, JT], fp32, tag="d2")
            sc.activation(d2[:], sd2, AF.Exp, scale=0.2)

            # ---- L. layer-2 attention ----
            acc2 = ps_attn.tile([128, QT * 17], fp32, tag="acc")
            for jt in range(JT):
                pt = ppool.tile([128, NQ], bf16, tag="pt")
                e = g if (jt % 8 == 4) else v
                e.tensor_scalar(pt[:], w2b[:], d2[:, jt:jt + 1],
                                b2[:, jt:jt + 1], OP.mult, OP.max)
                for qt in range(QT):
                    te.matmul(acc2[:, qt * 17:(qt + 1) * 17],
                              pt[:, qt * 128:(qt + 1) * 128],
                              hx2[:, jt * 17:(jt + 1) * 17],
                              start=(jt == 0), stop=(jt == JT - 1))

            # ---- M. normalize, elu, log_softmax, store (fp32 epilogue) ----
            r2 = workp.tile([128, QT], fp32, tag="r2")
            v.reciprocal(
                r2[:], acc2[:].rearrange("p (q c) -> p q c", c=17)[:, :, 16])
            o4 = workp.tile([128, QT * 16], fp32, tag="o4")
            for qt in range(QT):
                v.tensor_scalar(o4[:, qt * 16:(qt + 1) * 16],
                                acc2[:, qt * 17:qt * 17 + 16],
                                r2[:, qt:qt + 1], None, OP.mult)
            ex2 = workp.tile([128, QT * 16], fp32, tag="ex2")
            sc.activation(ex2[:], o4[:], AF.Exp)
            v.tensor_scalar(ex2[:], ex2[:], 1.0, 0.0, OP.subtract, OP.min)
            z2 = workp.tile([128, QT * 16], fp32, tag="z2")
            v.tensor_tensor(z2[:], o4[:], ex2[:], OP.max)
            fin = workp.tile([128, QT * 16], fp32, tag="fin")
            scratch = workp.tile([128, 16], fp32, tag="scr")
            for qt in range(QT):
                se = workp.tile([128, 1], fp32, tag=f"se{qt}", name=f"se{qt}")
                sc.activation(scratch[:], z2[:, qt * 16:(qt + 1) * 16],
                              AF.Exp, accum_out=se[:])
                lse = workp.tile([128, 1], fp32, tag=f"lse{qt}",
                                 name=f"lse{qt}")
                sc.activation(lse[:], se[:], AF.Ln)
                v.tensor_scalar(fin[:, qt * 16:(qt + 1) * 16],
                                z2[:, qt * 16:(qt + 1) * 16],
                                lse[:], None, OP.subtract)
            dma.dma_start(out.rearrange("(q p) c -> p q c", p=128),
                          fin[:].rearrange("p (q c) -> p q c", c=16))

    nc.finalize()
    return nc


def _get_compiled(no_cc=False):
    key = ("nc", no_cc)
    if key not in _CACHE:
        _CACHE[key] = _build_nc(no_cc=no_cc)
    return _CACHE[key]


def kernel(x, Wh, ah, Wo, ao):
    from concourse.bass_utils import run_bass_kernel_spmd
    import ml_dtypes

    bf = ml_dtypes.bfloat16
    nc = _get_compiled()
    x = np.asarray(x, np.float32)
    Wh = np.asarray(Wh, np.float32)
    ah = np.asarray(ah, np.float32)
    Wo = np.asarray(Wo, np.float32)
    ao = np.asarray(ao, np.float32)

    # host-side relayouts (no device math): head-major weight matrix,
    # score projection Wa = Wh @ a per head (src cols 0:8, dst cols 8:16)
    Whr = np.ascontiguousarray(
        Wh.transpose(1, 0, 2).reshape(NFEAT, HW)).astype(bf)   # [512, 512]
    Wa = np.zeros((NFEAT, 16), np.float32)
    for h in range(NHEADS):
        Wa[:, h] = Wh[h] @ ah[h, :NHID]
        Wa[:, 8 + h] = Wh[h] @ ah[h, NHID:]
    Wa = Wa.astype(bf)
    aodm = np.stack([ao[:NCLASS], ao[NCLASS:]])                # [2, 16]

    in_maps = []
    for i in range(NC):
        in_maps.append({
            "xT": np.ascontiguousarray(x[i * NQ:(i + 1) * NQ].T).astype(bf),
            "Whr": Whr, "Wa": Wa,
            "Wo": np.ascontiguousarray(Wo).astype(bf), "aod": aodm,
        })
    res = run_bass_kernel_spmd(nc, in_maps, list(range(NC)))
    return np.concatenate([res.results[i]["out"] for i in range(NC)], 0)


# revision 11
# speedup vs baseline: 1.5864x; 1.0992x over previous
"""GAT (2-layer, 8-head) fused Bass kernel for 8 trn2 NeuronCores.

Sharding: nodes (rows of x) split 512/core. Layer-1 h/scores computed locally
per core and AllGather'd (h+ones bf16 in two 4-head chunks for overlap,
scores fp32); each core computes its 512xN attention block for all 8 heads;
layer-2 projection + scores AllGather'd (fp32); each core computes its 512xN
layer-2 block and the final log_softmax rows.

Key algebra: with s_i = h_i . a_src, d_j = h_j . a_dst,
  exp(leakyrelu(s_i + d_j)) = max(exp(s_i)exp(d_j), exp(.2 s_i)exp(.2 d_j))
and softmax over j is invariant to any per-i scale, so the attention
numerator is P[k,q] = max(b_k, w_q * dd_k) with b = exp(d), w = exp(-.8 s),
dd = exp(.2 d).  P tiles are built key-major [128k, 512q] so w is the tensor
operand and b/dd are per-partition scalars; they are consumed as matmul
weights (lhsT) against the gathered h (rhs, 65 cols incl. a ones column for
the denominator), so attention output lands query-major [128q, 65] and needs
no transposes.  P builds are spread across DVE (tensor_scalar mult+max),
Activation (relu(dd*w - b), plus a rank-1 " + sum_k b_k h_k" PE fixup) and
GPSIMD.  s is computed as x @ (W a) with host-precomputed Wa.

In the no_cc (timing) variant each AllGather is modeled as one local copy of
the staged shard into block 0 of the gather buffer; the full-buffer readback
(identical to the real path) carries the ring-DMA cost.
"""

import numpy as np

N, NFEAT, NHID, NCLASS, NHEADS = 4096, 512, 64, 16, 8
NC = 8                      # cores
NQ = N // NC                # 512 own nodes per core
QT = NQ // 128              # 4 query tiles per core
JT = N // 128               # 32 key tiles
HW = NHID * NHEADS          # 512
HC = 4 * (NHID + 1)         # 260: AG1h chunk cols (4 heads x (64 h + ones))
AGC2 = 18                   # AG2: 16 outh + 1 ones + 1 sdst2


def _mk_assign(n_a, n_g):
    """Spread n_a Act and n_g GPSIMD P-tile builds evenly over 32 key tiles."""
    a = set(round(i * JT / n_a) % JT for i in range(n_a)) if n_a else set()
    gpos = []
    i = 0
    while len(gpos) < n_g:
        p = round(i * JT / n_g) % JT if n_g else 0
        while p in a or p in gpos:
            p = (p + 1) % JT
        gpos.append(p)
        i += 1
    return ["a" if j in a else "g" if j in gpos else "v" for j in range(JT)]


_ASSIGN = _mk_assign(6, 5)    # layer-1, per head
_ASSIGN2 = _mk_assign(8, 6)   # layer-2

_CACHE = {}


def _build_nc(no_cc=False):
    import concourse.bass as bass
    import concourse.bacc as bacc
    import concourse.mybir as mybir
    import concourse.tile as tile
    from concourse.masks import make_identity

    fp32 = mybir.dt.float32
    bf16 = mybir.dt.bfloat16
    AX = mybir.AxisListType.X
    OP = mybir.AluOpType
    AF = mybir.ActivationFunctionType

    nc = bacc.Bacc()
    xT = nc.declare_dram_parameter("xT", [NFEAT, NQ], bf16, isOutput=False)
    Whr = nc.declare_dram_parameter("Whr", [NFEAT, HW], bf16, isOutput=False)
    Wa = nc.declare_dram_parameter("Wa", [NFEAT, 16], bf16, isOutput=False)
    Wo = nc.declare_dram_parameter("Wo", [HW, NCLASS], bf16, isOutput=False)
    aod = nc.declare_dram_parameter("aod", [2, NCLASS], fp32, isOutput=False)
    out = nc.declare_dram_parameter("out", [NQ, NCLASS], fp32, isOutput=True)

    with tile.TileContext(nc) as tc:
        with (
            tc.tile_pool(name="const", bufs=1) as constp,
            tc.tile_pool(name="work", bufs=3) as workp,
            tc.tile_pool(name="pp", bufs=12) as ppool,
            tc.tile_pool(name="ps_attn", bufs=3, space="PSUM") as ps_attn,
            tc.tile_pool(name="ps_b", bufs=2, space="PSUM") as ps_b,
            tc.tile_pool(name="ps_t", bufs=2, space="PSUM") as ps_t,
            tc.tile_pool(name="ps_s", bufs=1, space="PSUM") as ps_s,
            tc.tile_pool(name="dram", bufs=1, space="DRAM") as dramp,
        ):
            v, sc, g, te, dma = nc.vector, nc.scalar, nc.gpsimd, nc.tensor, nc.sync

            ident = constp.tile([128, 128], bf16, tag="ident")
            make_identity(nc, ident[:])
            ident_f = constp.tile([128, 128], fp32, tag="ident_f")
            make_identity(nc, ident_f[:])
            # sel[k, h*128+m] = 1 iff k == h: one-hot row selector for
            # partition-broadcast matmuls (out = sel_h.T @ rows)
            sel = constp.tile([8, 8 * 128], bf16, tag="sel")
            g.memset(sel[:], 0.0)
            g.affine_select(
                out=sel[:].rearrange("k (h m) -> k h m", m=128),
                in_=sel[:].rearrange("k (h m) -> k h m", m=128),
                compare_op=OP.not_equal,
                fill=1.0, base=0, channel_multiplier=1,
                pattern=[[-1, 8], [0, 128]])
            ones_row = constp.tile([1, 128], bf16, tag="ones_row")
            g.memset(ones_row[:], 1.0)

            # ---- A. param loads (bf16, batched) ----
            xt = constp.tile([128, 4 * NQ], bf16, tag="xt")
            dma.dma_start(xt[:].rearrange("p (k q) -> p k q", k=4),
                          xT.rearrange("(k p) q -> p k q", p=128))
            wa = constp.tile([128, 64], bf16, tag="wa")
            dma.dma_start(wa[:].rearrange("p (k c) -> p k c", k=4),
                          Wa.rearrange("(k p) c -> p k c", p=128))
            wh = constp.tile([128, 4 * HW], bf16, tag="wh")
            dma.dma_start(wh[:].rearrange("p (k c) -> p k c", k=4),
                          Whr.rearrange("(k p) c -> p k c", p=128))
            wo = constp.tile([128, 64], bf16, tag="wo")
            dma.dma_start(wo[:].rearrange("p (k c) -> p k c", k=4),
                          Wo.rearrange("(k p) c -> p k c", p=128))
            aos_b = constp.tile([128, 16], fp32, tag="aos_b")
            dma.dma_start(aos_b[:], aod[0:1, :].to_broadcast((128, 16)))
            aod_b = constp.tile([128, 16], fp32, tag="aod_b")
            dma.dma_start(aod_b[:], aod[1:2, :].to_broadcast((128, 16)))

            ag1s_in = dramp.tile([NQ, 16], fp32, tag="ag1s_in")
            ag1s_out = dramp.tile([N, 16], fp32, tag="ag1s_out",
                                  addr_space="Local" if no_cc else "Shared")
            agh_in = [dramp.tile([NQ, HC], bf16, tag=f"agh_in{c}",
                                 name=f"agh_in{c}") for c in range(2)]
            agh_out = [dramp.tile([N, HC], bf16, tag=f"agh_out{c}",
                                  name=f"agh_out{c}",
                                  addr_space="Local" if no_cc else "Shared")
                       for c in range(2)]
            ag2_in = dramp.tile([NQ, AGC2], fp32, tag="ag2_in")
            ag2_out = dramp.tile([N, AGC2], fp32, tag="ag2_out",
                                 addr_space="Local" if no_cc else "Shared")

            # ---- B1. s_own = x @ Wa (own scores), staged + gathered first
            s_own = constp.tile([128, QT * 16], fp32, tag="s_own")
            for qt in range(QT):
                pss = ps_s.tile([128, 16], fp32, tag="sp")
                for k in range(4):
                    te.matmul(pss[:],
                              xt[:, k * NQ + qt * 128:k * NQ + (qt + 1) * 128],
                              wa[:, k * 16:(k + 1) * 16],
                              start=(k == 0), stop=(k == 3))
                v.tensor_copy(s_own[:, qt * 16:(qt + 1) * 16], pss[:])
            dma.dma_start(ag1s_in.rearrange("(q p) c -> p q c", p=128),
                          s_own[:].rearrange("p (q c) -> p q c", c=16))
            if no_cc:
                dma.dma_start(ag1s_out[0:NQ, :], ag1s_in[:])
            else:
                g.collective_compute(
                    "AllGather", OP.bypass,
                    ins=[ag1s_in.opt()], outs=[ag1s_out.opt()],
                    replica_groups=[list(range(NC))])

            # ---- B2. h_own per query tile; stage + gather AG1h chunks ----
            stg = [constp.tile([128, QT * HC], bf16, tag=f"stg{c}",
                               name=f"stg{c}") for c in range(2)]
            for c in range(2):
                g.memset(
                    stg[c][:].rearrange("p (q h c) -> p q h c", h=4, c=65)
                    [:, :, :, 64:65], 1.0)
            for qt in range(QT):
                ps_h = ps_b.tile([128, HW], fp32, tag="bp")
                for k in range(4):
                    te.matmul(ps_h[:],
                              xt[:, k * NQ + qt * 128:k * NQ + (qt + 1) * 128],
                              wh[:, k * HW:(k + 1) * HW],
                              start=(k == 0), stop=(k == 3))
                for c in range(2):
                    sc.copy(
                        stg[c][:, qt * HC:(qt + 1) * HC]
                        .rearrange("p (h c) -> p h c", c=65)[:, :, 0:64],
                        ps_h[:, c * 256:(c + 1) * 256]
                        .rearrange("p (h c) -> p h c", c=64))
            for c in range(2):
                dma.dma_start(agh_in[c].rearrange("(q p) c -> p q c", p=128),
                              stg[c][:].rearrange("p (q c) -> p q c", c=HC))
                if no_cc:
                    dma.dma_start(agh_out[c][0:NQ, :], agh_in[c][:])
                else:
                    g.collective_compute(
                        "AllGather", OP.bypass,
                        ins=[agh_in[c].opt()], outs=[agh_out[c].opt()],
                        replica_groups=[list(range(NC))])

            # ---- C. w panel: exp(-0.8 s_src) bcast to [128, NQ] per head ----
            s_f = constp.tile([16, NQ], fp32, tag="s_f")
            for qt in range(QT):
                tp = ps_t.tile([16, 128], fp32, tag="tp", name="tp_s")
                te.transpose(tp[:], s_own[:, qt * 16:(qt + 1) * 16],
                             ident_f[:])
                (v.tensor_copy if qt % 2 else sc.copy)(
                    s_f[:, qt * 128:(qt + 1) * 128], tp[:])
            w8 = constp.tile([8, NQ], bf16, tag="w8")
            sc.activation(w8[:], s_f[0:8, :], AF.Exp, scale=-0.8)
            wb = []
            for h in range(NHEADS):
                bp = ps_b.tile([128, NQ], fp32, tag="bp")
                te.matmul(bp[:], sel[:, h * 128:(h + 1) * 128], w8[:],
                          start=True, stop=True)
                t = constp.tile([128, NQ], bf16, tag=f"wb{h}", name=f"wb{h}")
                (v.tensor_copy if h % 2 else sc.copy)(t[:], bp[:])
                wb.append(t)

            # ---- D. gathered score panels: b = exp(d), dd = exp(.2 d) ----
            sd_pan = constp.tile([128, JT * 16], fp32, tag="sd_pan")
            dma.dma_start(sd_pan[:].rearrange("p (t c) -> p t c", c=16),
                          ag1s_out.rearrange("(t p) c -> p t c", p=128))
            dstv = sd_pan[:].rearrange("p (t c) -> p t c", c=16)[:, :, 8:16]
            b_all = constp.tile([128, JT * NHEADS], fp32, tag="b_all")
            sc.activation(b_all[:].rearrange("p (t h) -> p t h", h=8), dstv,
                          AF.Exp)
            d_all = constp.tile([128, JT * NHEADS], fp32, tag="d_all")
            sc.activation(d_all[:].rearrange("p (t h) -> p t h", h=8), dstv,
                          AF.Exp, scale=0.2)
            negb = constp.tile([128, JT * NHEADS], fp32, tag="negb")
            v.tensor_scalar(negb[:], b_all[:], -1.0, None, OP.mult)
            b_bf = constp.tile([128, JT * NHEADS], bf16, tag="b_bf")
            g.tensor_copy(b_bf[:], b_all[:])

            # ---- E. hx loads (full gathered buffer, 2 halves per chunk) ----
            hxc = []
            for c in range(2):
                hx = constp.tile([128, JT * HC], bf16, tag=f"hx{c}",
                                 name=f"hx{c}")
                for half in range(2):
                    rows = agh_out[c][half * (N // 2):(half + 1) * (N // 2), :]
                    dma.dma_start(
                        hx[:, half * 16 * HC:(half + 1) * 16 * HC]
                        .rearrange("p (t c) -> p t c", c=HC),
                        rows.rearrange("(t p) c -> p t c", p=128))
                hxc.append(hx)

            # ---- F. layer-1 attention (flipped: out is query-major) ----
            n_act = sum(1 for a in _ASSIGN if a == "a")
            xr = [constp.tile([128, HW], bf16, tag=f"xr{qt}", name=f"xr{qt}")
                  for qt in range(QT)]
            for h in range(NHEADS):
                hx = hxc[h // 4]
                coff = (h % 4) * 65
                acc = ps_attn.tile([128, QT * 65], fp32, tag="acc")
                accC = None
                if n_act:
                    accC = ps_s.tile([1, 65], fp32, tag="sp", name="accC")
                seen_act = 0
                for jt in range(JT):
                    col = jt * NHEADS + h
                    eng = _ASSIGN[jt]
                    pt = ppool.tile([128, NQ], bf16, tag="pt")
                    if eng == "a":
                        sc.activation(pt[:], wb[h][:], AF.Relu,
                                      bias=negb[:, col:col + 1],
                                      scale=d_all[:, col:col + 1])
                        te.matmul(accC[:], b_bf[:, col:col + 1],
                                  hx[:, jt * HC + coff:jt * HC + coff + 65],
                                  start=(seen_act == 0),
                                  stop=(seen_act == n_act - 1))
                        seen_act += 1
                    else:
                        e = v if eng == "v" else g
                        e.tensor_scalar(pt[:], wb[h][:],
                                        d_all[:, col:col + 1],
                                        b_all[:, col:col + 1],
                                        OP.mult, OP.max)
                    for qt in range(QT):
                        te.matmul(acc[:, qt * 65:(qt + 1) * 65],
                                  pt[:, qt * 128:(qt + 1) * 128],
                                  hx[:, jt * HC + coff:jt * HC + coff + 65],
                                  start=(jt == 0),
                                  stop=(jt == JT - 1 and n_act == 0))
                if n_act:
                    crow = workp.tile([1, 65], bf16, tag="crow")
                    v.tensor_copy(crow[:], accC[:])
                    for qt in range(QT):
                        te.matmul(acc[:, qt * 65:(qt + 1) * 65], ones_row[:],
                                  crow[:], start=False, stop=True)
                # normalize: den fp32 from PSUM, feat via bf16 copy
                racc = workp.tile([128, QT], fp32, tag="racc")
                v.reciprocal(
                    racc[:],
                    acc[:].rearrange("p (q c) -> p q c", c=65)[:, :, 64])
                ab = workp.tile([128, QT * 65], bf16, tag="ab")
                sc.copy(ab[:], acc[:])
                for qt in range(QT):
                    v.tensor_scalar(xr[qt][:, h * 64:(h + 1) * 64],
                                    ab[:, qt * 65:qt * 65 + 64],
                                    racc[:, qt:qt + 1], None, OP.mult)

            # ---- G. elu -> xc (bf16) -> xcT ----
            xc = [constp.tile([128, HW], bf16, tag=f"xc{qt}", name=f"xc{qt}")
                  for qt in range(QT)]
            xcT = [constp.tile([128, NQ], bf16, tag=f"xcT{k}", name=f"xcT{k}")
                   for k in range(4)]
            for qt in range(QT):
                ex = workp.tile([128, HW], bf16, tag="ex")
                sc.activation(ex[:], xr[qt][:], AF.Exp)
                v.tensor_scalar(ex[:], ex[:], 1.0, 0.0, OP.subtract, OP.min)
                v.tensor_tensor(xc[qt][:], xr[qt][:], ex[:], OP.max)
            for qt in range(QT):
                for fb in range(4):
                    tp = ps_t.tile([128, 128], bf16, tag="tp")
                    te.transpose(tp[:], xc[qt][:, fb * 128:(fb + 1) * 128],
                                 ident[:])
                    e = v.tensor_copy if (qt * 4 + fb) % 2 else sc.copy
                    e(xcT[fb][:, qt * 128:(qt + 1) * 128], tp[:])

            # ---- H. layer-2 projection, scores, stage AG2 ----
            stg2 = constp.tile([128, QT * AGC2], fp32, tag="stg2")
            g.memset(
                stg2[:].rearrange("p (q c) -> p q c", c=AGC2)[:, :, 16:17],
                1.0)
            w2tmp = workp.tile([128, QT], fp32, tag="w2tmp")
            for qt in range(QT):
                pso = ps_s.tile([128, 16], fp32, tag="sp")
                for k in range(4):
                    te.matmul(pso[:], xcT[k][:, qt * 128:(qt + 1) * 128],
                              wo[:, k * 16:(k + 1) * 16],
                              start=(k == 0), stop=(k == 3))
                v.tensor_copy(stg2[:, qt * AGC2:qt * AGC2 + 16], pso[:])
                tmp = workp.tile([128, 16], fp32, tag="sdtmp")
                v.tensor_tensor(tmp[:], pso[:], aod_b[:], OP.mult)
                v.tensor_reduce(stg2[:, qt * AGC2 + 17:qt * AGC2 + 18],
                                tmp[:], AX, OP.add)
                v.tensor_tensor(tmp[:], pso[:], aos_b[:], OP.mult)
                v.tensor_reduce(w2tmp[:, qt:qt + 1], tmp[:], AX, OP.add)
            dma.dma_start(ag2_in.rearrange("(q p) c -> p q c", p=128),
                          stg2[:].rearrange("p (q c) -> p q c", c=AGC2))
            if no_cc:
                dma.dma_start(ag2_out[0:NQ, :], ag2_in[:])
            else:
                g.collective_compute(
                    "AllGather", OP.bypass,
                    ins=[ag2_in.opt()], outs=[ag2_out.opt()],
                    replica_groups=[list(range(NC))])

            # ---- I. w2 panel ----
            w2e = workp.tile([128, QT], fp32, tag="w2e")
            sc.activation(w2e[:], w2tmp[:], AF.Exp, scale=-0.8)
            w2tp = ps_t.tile([QT, 128], fp32, tag="tp", name="w2tp")
            te.transpose(w2tp[:], w2e[:], ident_f[:])
            w2s = workp.tile([QT, 128], bf16, tag="w2s")
            v.tensor_copy(w2s[:], w2tp[:])
            w2b = constp.tile([128, NQ], bf16, tag="w2b")
            for qt in range(QT):
                w2ps = ps_t.tile([128, 128], fp32, tag="tp", name="w2ps")
                te.matmul(w2ps[:], sel[0:QT, qt * 128:(qt + 1) * 128],
                          w2s[:], start=True, stop=True)
                (sc.copy if qt % 2 else v.tensor_copy)(
                    w2b[:, qt * 128:(qt + 1) * 128], w2ps[:])

            # ---- J. layer-2 panels ----
            hx2f = constp.tile([128, JT * AGC2], fp32, tag="hx2f")
            dma.dma_start(hx2f[:].rearrange("p (t c) -> p t c", c=AGC2),
                          ag2_out.rearrange("(t p) c -> p t c", p=128))
            hx2 = constp.tile([128, JT * 17], bf16, tag="hx2")
            sc.copy(hx2[:].rearrange("p (t c) -> p t c", c=17),
                    hx2f[:].rearrange("p (t c) -> p t c", c=AGC2)[:, :, 0:17])
            sd2 = hx2f[:].rearrange("p (t c) -> p t c", c=AGC2)[:, :, 17]
            b2 = constp.tile([128, JT], fp32, tag="b2")
            sc.activation(b2[:], sd2, AF.Exp)
            d2 = constp.tile([128, JT], fp32, tag="d2")
            sc.activation(d2[:], sd2, AF.Exp, scale=0.2)
            negb2 = constp.tile([128, JT], fp32, tag="negb2")
            v.tensor_scalar(negb2[:], b2[:], -1.0, None, OP.mult)
            b2bf = constp.tile([128, JT], bf16, tag="b2bf")
            g.tensor_copy(b2bf[:], b2[:])

            # ---- K. layer-2 attention ----
            n_act2 = sum(1 for a in _ASSIGN2 if a == "a")
            acc2 = ps_attn.tile([128, QT * 17], fp32, tag="acc")
            accC2 = ps_s.tile([1, 17], fp32, tag="sp", name="accC2")
            seen_act = 0
            for jt in range(JT):
                eng = _ASSIGN2[jt]
                pt = ppool.tile([128, NQ], bf16, tag="pt")
                if eng == "a":
                    sc.activation(pt[:], w2b[:], AF.Relu,
                                  bias=negb2[:, jt:jt + 1],
                                  scale=d2[:, jt:jt + 1])
                    te.matmul(accC2[:], b2bf[:, jt:jt + 1],
                              hx2[:, jt * 17:(jt + 1) * 17],
                              start=(seen_act == 0),
                              stop=(seen_act == n_act2 - 1))
                    seen_act += 1
                else:
                    e = v if eng == "v" else g
                    e.tensor_scalar(pt[:], w2b[:], d2[:, jt:jt + 1],
                                    b2[:, jt:jt + 1], OP.mult, OP.max)
                for qt in range(QT):
                    te.matmul(acc2[:, qt * 17:(qt + 1) * 17],
                              pt[:, qt * 128:(qt + 1) * 128],
                              hx2[:, jt * 17:(jt + 1) * 17],
                              start=(jt == 0),
                              stop=(jt == JT - 1 and n_act2 == 0))
            if n_act2:
                crow2 = workp.tile([1, 17], bf16, tag="crow2")
                v.tensor_copy(crow2[:], accC2[:])
                for qt in range(QT):
                    te.matmul(acc2[:, qt * 17:(qt + 1) * 17], ones_row[:],
                              crow2[:], start=False, stop=True)

            # ---- L. normalize, elu, log_softmax, store (fp32 epilogue) ----
            r2 = workp.tile([128, QT], fp32, tag="r2")
            v.reciprocal(
                r2[:], acc2[:].rearrange("p (q c) -> p q c", c=17)[:, :, 16])
            o4 = workp.tile([128, QT * 16], fp32, tag="o4")
            for qt in range(QT):
                v.tensor_scalar(o4[:, qt * 16:(qt + 1) * 16],
                                acc2[:, qt * 17:qt * 17 + 16],
                                r2[:, qt:qt + 1], None, OP.mult)
            ex2 = workp.tile([128, QT * 16], fp32, tag="ex2")
            sc.activation(ex2[:], o4[:], AF.Exp)
            v.tensor_scalar(ex2[:], ex2[:], 1.0, 0.0, OP.subtract, OP.min)
            z2 = workp.tile([128, QT * 16], fp32, tag="z2")
            v.tensor_tensor(z2[:], o4[:], ex2[:], OP.max)
            # batched Exp(+accum) then a single Ln avoids act-table thrash
            se4 = workp.tile([128, QT], fp32, tag="se4")
            scratch = workp.tile([128, 16], fp32, tag="scr")
            for qt in range(QT):
                sc.activation(scratch[:], z2[:, qt * 16:(qt + 1) * 16],
                              AF.Exp, accum_out=se4[:, qt:qt + 1])
            lse4 = workp.tile([128, QT], fp32, tag="lse4")
            sc.activation(lse4[:], se4[:], AF.Ln)
            fin = workp.tile([128, QT * 16], fp32, tag="fin")
            for qt in range(QT):
                v.tensor_scalar(fin[:, qt * 16:(qt + 1) * 16],
                                z2[:, qt * 16:(qt + 1) * 16],
                                lse4[:, qt:qt + 1], None, OP.subtract)
            dma.dma_start(out.rearrange("(q p) c -> p q c", p=128),
                          fin[:].rearrange("p (q c) -> p q c", c=16))

    nc.finalize()
    return nc


def _get_compiled(no_cc=False):
    key = ("nc", no_cc)
    if key not in _CACHE:
        _CACHE[key] = _build_nc(no_cc=no_cc)
    return _CACHE[key]


def kernel(x, Wh, ah, Wo, ao):
    from concourse.bass_utils import run_bass_kernel_spmd
    import ml_dtypes

    bf = ml_dtypes.bfloat16
    nc = _get_compiled()
    x = np.asarray(x, np.float32)
    Wh = np.asarray(Wh, np.float32)
    ah = np.asarray(ah, np.float32)
    Wo = np.asarray(Wo, np.float32)
    ao = np.asarray(ao, np.float32)

    # host-side relayouts (no device math): head-major weight matrix,
    # score projection Wa = Wh @ a per head (src cols 0:8, dst cols 8:16)
    Whr = np.ascontiguousarray(
        Wh.transpose(1, 0, 2).reshape(NFEAT, HW)).astype(bf)   # [512, 512]
    Wa = np.zeros((NFEAT, 16), np.float32)
    for h in range(NHEADS):
        Wa[:, h] = Wh[h] @ ah[h, :NHID]
        Wa[:, 8 + h] = Wh[h] @ ah[h, NHID:]
    Wa = Wa.astype(bf)
    aodm = np.stack([ao[:NCLASS], ao[NCLASS:]])                # [2, 16]

    in_maps = []
    for i in range(NC):
        in_maps.append({
            "xT": np.ascontiguousarray(x[i * NQ:(i + 1) * NQ].T).astype(bf),
            "Whr": Whr, "Wa": Wa,
            "Wo": np.ascontiguousarray(Wo).astype(bf), "aod": aodm,
        })
    res = run_bass_kernel_spmd(nc, in_maps, list(range(NC)))
    return np.concatenate([res.results[i]["out"] for i in range(NC)], 0)
